# revision 12
# baseline (speedup 1.0000x reference)
"""LUT-dense kernel for Trainium2 (8 NeuronCores).

out[m, n] = sum_k LUT[a[m, k], w[n, k]] + bias[n]

Formulated as an exact bf16 matmul with contraction over (k, p):
    out[m, n] = sum_{k, p} [a[m,k] == p] * LUT[p, w[n,k]]

Per k the PE consumes two 128-partition contraction chunks (p in [0,128) and
[128,256)):
  - rhs H_k[p, n] = LUT[p, w[n,k]] comes from SWDGE dma_gather(transpose=True)
    over lut.T rows (row j = LUT[:, j]) indexed by the w-codes; the gather
    transpose lands each 256-wide row across partitions = code-on-partitions.
    The gather ucode tops out between 512 and 1024 indices per instruction on
    this silicon, so it's one 512-index gather per k.
  - lhsT A1h_k[p, m] = [a[m,k] == p] is built on chip: a broadcast DMA
    replicates the a^T code row across all 128 partitions (stride-0 HBM read),
    then a DVE tensor_scalar is_equal against a per-partition index column
    produces the one-hot. This keeps POOL/SWDGE off the critical path.

Accumulation: 4 persistent PSUM banks (one per 128-row m-tile) accumulate all
K*2 chunk matmuls; the bias is added by a final rank-1 matmul (ones x bias
row), then VectorE evicts PSUM -> SBUF -> HBM.

Sharding: 2x4 (M x N) grid over 8 cores; each core computes a 512x512 block.
"""

import numpy as np
import ml_dtypes

import concourse.bass as bass
import concourse.mybir as mybir
from concourse.tile import TileContext
from concourse.bass_utils import run_bass_kernel_spmd
from concourse.library_config import mlp
from concourse.library_overlay import lower_extended_insts

M, K, N = 1024, 2048, 2048
GRID_M, GRID_N = 2, 4
N_CORES = GRID_M * GRID_N

BF16 = mybir.dt.bfloat16
F32 = mybir.dt.float32
I16 = mybir.dt.int16

MAX_GATHER_IDX = 512  # HW dma_gather limit: 512 ok, 1024 wedges the core


def _walrus_compat(nc):
    """Two workarounds for the nix walrus in this image:
    1. It can't encode EVENT_SEMAPHORE_RANGE_CLEAR (ISA wrong length) —
       replace each with per-sem EVENT_SEMAPHORE sem-wr-imm 0 writes.
    2. It allows only one sync-wait per instruction — hoist extra waits
       onto preceding same-engine EVENT_SEMAPHORE carriers.
    """
    n_rc, n_split = 0, 0
    for f in nc.m.functions:
        for b in f.blocks:
            insts = list(b.instructions)
            new = []
            for i in insts:
                if type(i).__name__ == "InstISA" and getattr(i, "isa_opcode", 0) == 176:
                    d = i.ant_dict
                    first, last = d["range_first"], d["range_last"]
                    si = i.sync_info
                    waits = list(si.on_wait) if si is not None else []
                    updates = list(si.on_update) if si is not None else []
                    reps = []
                    for s in range(first, last + 1):
                        upds = [mybir.SyncUpdate(
                            sync_type="semaphore", id=s,
                            update_mode="sem-wr-imm", update_value=0,
                            ant_name=f"clr{s}")]
                        reps.append(mybir.InstEventSemaphore(
                            name=f"{i.name}_c{s}", engine=i.engine,
                            ins=[], outs=[],
                            sync_info=mybir.SyncInfo(on_wait=[], on_update=upds),
                        ))
                    if waits:
                        si0 = reps[0].sync_info
                        reps[0].sync_info = mybir.SyncInfo(
                            on_wait=waits, on_update=list(si0.on_update))
                    if updates:
                        sil = reps[-1].sync_info
                        reps[-1].sync_info = mybir.SyncInfo(
                            on_wait=list(sil.on_wait),
                            on_update=list(sil.on_update) + updates)
                    new.extend(reps)
                    n_rc += 1
                else:
                    new.append(i)
            out = []
            for i in new:
                si = i.sync_info
                if si is not None and len(si.on_wait) > 1:
                    waits = list(si.on_wait)
                    for j, w in enumerate(waits[:-1]):
                        out.append(mybir.InstEventSemaphore(
                            name=f"{i.name}_w{j}", engine=i.engine,
                            ins=[], outs=[],
                            sync_info=mybir.SyncInfo(on_wait=[w], on_update=[]),
                        ))
                    i.sync_info = mybir.SyncInfo(
                        on_wait=[waits[-1]], on_update=list(si.on_update))
                    n_split += 1
                out.append(i)
            b.instructions.clear()
            for x in out:
                b.instructions.append(x)
    return n_rc, n_split


def build_nc(Ms, Ns, Kd, BLK):
    """One SPMD program; per-core data differs via in_maps."""
    NBLK = Kd // BLK
    MT = Ms // 128
    IDXW = Ns // 16  # idx columns per k for the W-side gather

    nc = bass.Bass()
    widx = nc.declare_dram_parameter("widx", [NBLK, 128, BLK * IDXW], I16, False)
    abf = nc.declare_dram_parameter("abf", [Kd, Ms], BF16, False)
    lutT = nc.declare_dram_parameter("lutT", [256, 256], BF16, False)
    pcol = nc.declare_dram_parameter("pcol", [128, 2], F32, False)
    biasr = nc.declare_dram_parameter("biasr", [1, Ns], F32, False)
    ones = nc.declare_dram_parameter("ones", [1, 128], F32, False)
    out_d = nc.declare_dram_parameter("out", [Ms, Ns], F32, isOutput=True)

    with TileContext(nc) as tc:
        with (
            tc.tile_pool(name="const", bufs=1) as constp,
            tc.tile_pool(name="idx", bufs=3) as idxp,
            tc.tile_pool(name="arep", bufs=3) as arepp,
            tc.tile_pool(name="gath", bufs=8) as gathp,
            tc.tile_pool(name="psum", bufs=1, space="PSUM") as psump,
            tc.tile_pool(name="outs", bufs=2) as outp,
        ):
            nc.gpsimd.load_library(mlp)
            nidx_reg = nc.gpsimd.to_reg(Ns)

            ones_t = constp.tile([1, 128], F32, tag="ones")
            nc.sync.dma_start(out=ones_t, in_=ones[:, :])
            bias_t = constp.tile([1, Ns], F32, tag="bias")
            nc.sync.dma_start(out=bias_t, in_=biasr[:, :])
            pcol_t = constp.tile([128, 2], F32, tag="pcol")
            nc.sync.dma_start(out=pcol_t, in_=pcol[:, :])

            psums = [
                psump.tile([128, Ns], F32, tag=f"ps{mt}", name=f"ps{mt}")
                for mt in range(MT)
            ]

            for b in range(NBLK):
                wt = idxp.tile([128, BLK * IDXW], I16, tag="widx")
                nc.sync.dma_start(out=wt, in_=widx[b])
                arep = arepp.tile([128, BLK, Ms], BF16, tag="arep")
                nc.sync.dma_start(
                    out=arep,
                    in_=abf[b * BLK:(b + 1) * BLK, :].partition_broadcast(128),
                )
                for kl in range(BLK):
                    k = b * BLK + kl
                    hg = gathp.tile([128, 2, Ns], BF16, tag="hg")
                    nc.gpsimd.dma_gather(
                        hg[:], lutT[:, :], wt[:, kl * IDXW:(kl + 1) * IDXW],
                        Ns, nidx_reg, 256, transpose=True, queue_num=0,
                    )
                    aoh = gathp.tile([128, 2, Ms], BF16, tag="aoh")
                    for half in range(2):
                        nc.vector.tensor_scalar(
                            aoh[:, half, :], arep[:, kl, :],
                            pcol_t[:, half:half + 1], None,
                            op0=mybir.AluOpType.is_equal,
                        )
                    for half in range(2):
                        rhs = hg[:, half, :]
                        for mt in range(MT):
                            lhsT = aoh[:, half, mt * 128:(mt + 1) * 128]
                            nc.tensor.matmul(
                                psums[mt], lhsT, rhs,
                                start=(k == 0 and half == 0),
                                stop=False,
                            )

            for mt in range(MT):
                nc.tensor.matmul(psums[mt], ones_t, bias_t, start=False, stop=True)

            for mt in range(MT):
                ot = outp.tile([128, Ns], F32, tag="ot")
                nc.vector.tensor_copy(ot, psums[mt])
                nc.sync.dma_start(out=out_d[mt * 128:(mt + 1) * 128, :], in_=ot)
    return nc


def wrap_idx_per_k(codes_T, blk):
    """codes_T: [K, S] ints. Per-k wrapped indices (idx i -> [i % 16, i // 16]),
    replicated to all 8 Q7 core groups, grouped in blocks of `blk` k's along
    the free dim. Returns [K//blk, 128, blk*S//16] int16."""
    Kd, S = codes_T.shape
    w = codes_T.reshape(Kd, -1, 16).transpose(0, 2, 1)      # [K, 16, S/16]
    w = np.tile(w, (1, 8, 1))                               # [K, 128, S/16]
    w = w.reshape(Kd // blk, blk, 128, S // 16).transpose(0, 2, 1, 3)
    return np.ascontiguousarray(w).reshape(
        Kd // blk, 128, blk * (S // 16)).astype(np.int16)


def make_in_maps(input, weight, bias, lut, Ms, Ns, Kd, BLK, grid_n):
    a = np.clip(np.asarray(input).astype(np.int32), 0, 255)
    w = np.clip(np.asarray(weight).astype(np.int32), 0, 255)
    lutT = np.ascontiguousarray(np.asarray(lut).astype(np.float32).T).astype(
        ml_dtypes.bfloat16
    )
    pcol = np.stack([np.arange(128), np.arange(128, 256)], axis=1).astype(
        np.float32
    )
    ones = np.ones((1, 128), np.float32)
    bias = np.asarray(bias).astype(np.float32)

    n_cores = (a.shape[0] // Ms) * grid_n
    in_maps = []
    for c in range(n_cores):
        mi, nj = c // grid_n, c % grid_n
        aT = a[mi * Ms:(mi + 1) * Ms, :].T        # [K, Ms]
        wT = w[nj * Ns:(nj + 1) * Ns, :].T        # [K, Ns]
        in_maps.append({
            "widx": wrap_idx_per_k(wT, BLK),
            "abf": np.ascontiguousarray(aT).astype(np.float32).astype(
                ml_dtypes.bfloat16),
            "lutT": lutT,
            "pcol": pcol,
            "biasr": bias[nj * Ns:(nj + 1) * Ns].reshape(1, Ns),
            "ones": ones,
        })
    return in_maps


_NC_CACHE = {}


def _get_nc(Ms, Ns, Kd, BLK, compat=True):
    key = (Ms, Ns, Kd, BLK)
    if key not in _NC_CACHE:
        nc = build_nc(Ms, Ns, Kd, BLK)
        if compat:
            lower_extended_insts(nc)
            _walrus_compat(nc)
        _NC_CACHE[key] = nc
    return _NC_CACHE[key]


def kernel(input, weight, bias, lut, BLK=8, trace=False, trace_kwargs=None):
    Ms, Ns = M // GRID_M, N // GRID_N
    nc = _get_nc(Ms, Ns, K, BLK)
    in_maps = make_in_maps(input, weight, bias, lut, Ms, Ns, K, BLK, GRID_N)
    res = run_bass_kernel_spmd(
        nc, in_maps, list(range(N_CORES)), trace=trace,
        trace_kwargs=trace_kwargs or {},
    )
    out = np.empty((M, N), np.float32)
    for c in range(N_CORES):
        mi, nj = c // GRID_N, c % GRID_N
        out[mi * Ms:(mi + 1) * Ms, nj * Ns:(nj + 1) * Ns] = res.results[c]["out"]
    if trace:
        kernel.last_exec_time_ns = res.exec_time_ns
    return out


kernel.last_exec_time_ns = None


# revision 15
# speedup vs baseline: 1.9876x; 1.9876x over previous
"""LUT-dense kernel for Trainium2 (8 NeuronCores).

out[m, n] = sum_k LUT[a[m, k], w[n, k]] + bias[n]

Exact bf16 PE formulation with contraction over (k, p):
    out[m, n] = sum_{k, p} [a[m,k] == p] * LUT[p, w[n,k]]

Per k the PE consumes two 128-partition contraction chunks (p in [0,128),
[128,256)):
  - lhsT A1h_k[p, m] = [a[m,k] == p]: a broadcast DMA replicates the a^T code
    row across all 128 partitions (stride-0 HBM read), then DVE tensor_tensor
    is_equal against a materialized partition-index tile gives the one-hot
    (bf16, 2x DVE mode).
  - rhs H_k[p, n] = LUT[p, w[n,k]] comes from one of two producers, balanced
    so neither POOL nor PE is oversubscribed:
      * k <  KSPLIT: stage-1 PE matmul H_k = L^T-chunks @ W1h_k, where the
        one-hot W1h_k is built exactly like the A-side (broadcast + is_equal);
        ScalarE evicts the PSUM result to bf16 SBUF.
      * k >= KSPLIT: SWDGE dma_gather(transpose=True) over lut.T rows (row j
        = LUT[:, j]) indexed by w-codes; the transpose lands each 256-wide row
        across partitions. Two k's are packed per gather (512 indices — the
        HW limit; 1024 wedges the core) to amortize the ~1us fixed Q7
        descriptor-generation cost, which is the scarce resource (~9.4ns per
        gathered row, measured).

Accumulation: 4 persistent PSUM banks, each holding two 256-wide m-tile
slots (8 m-tiles of the 1024-row shard); bias lands via a final rank-1
matmul (ones x bias row); VectorE evicts PSUM -> SBUF -> HBM.

Sharding: 1x8 N-split — each core computes the full 1024 rows x 256 output
columns. N-only sharding avoids duplicating the per-(k,n) gather work that
an M-split would replicate.
"""

import numpy as np
import ml_dtypes

import concourse.bass as bass
import concourse.mybir as mybir
from concourse.tile import TileContext
from concourse.bass_utils import run_bass_kernel_spmd
from concourse.library_config import mlp
from concourse.library_overlay import lower_extended_insts

M, K, N = 1024, 2048, 2048
GRID_N = 8
N_CORES = GRID_N

BF16 = mybir.dt.bfloat16
F32 = mybir.dt.float32
I16 = mybir.dt.int16

KSPLIT = 512          # k < KSPLIT: stage-1 PE H-build; k >= KSPLIT: dma_gather
MAX_GATHER_IDX = 512  # HW dma_gather limit: 512 ok, 1024 wedges the core


def _walrus_compat(nc):
    """Two workarounds for the nix walrus in this image:
    1. It can't encode EVENT_SEMAPHORE_RANGE_CLEAR (ISA wrong length) —
       replace each with per-sem EVENT_SEMAPHORE sem-wr-imm 0 writes.
    2. It allows only one sync-wait per instruction — hoist extra waits
       onto preceding same-engine EVENT_SEMAPHORE carriers.
    """
    n_rc, n_split = 0, 0
    for f in nc.m.functions:
        for b in f.blocks:
            insts = list(b.instructions)
            new = []
            for i in insts:
                if type(i).__name__ == "InstISA" and getattr(i, "isa_opcode", 0) == 176:
                    d = i.ant_dict
                    first, last = d["range_first"], d["range_last"]
                    si = i.sync_info
                    waits = list(si.on_wait) if si is not None else []
                    updates = list(si.on_update) if si is not None else []
                    reps = []
                    for s in range(first, last + 1):
                        upds = [mybir.SyncUpdate(
                            sync_type="semaphore", id=s,
                            update_mode="sem-wr-imm", update_value=0,
                            ant_name=f"clr{s}")]
                        reps.append(mybir.InstEventSemaphore(
                            name=f"{i.name}_c{s}", engine=i.engine,
                            ins=[], outs=[],
                            sync_info=mybir.SyncInfo(on_wait=[], on_update=upds),
                        ))
                    if waits:
                        si0 = reps[0].sync_info
                        reps[0].sync_info = mybir.SyncInfo(
                            on_wait=waits, on_update=list(si0.on_update))
                    if updates:
                        sil = reps[-1].sync_info
                        reps[-1].sync_info = mybir.SyncInfo(
                            on_wait=list(sil.on_wait),
                            on_update=list(sil.on_update) + updates)
                    new.extend(reps)
                    n_rc += 1
                else:
                    new.append(i)
            out = []
            for i in new:
                si = i.sync_info
                if si is not None and len(si.on_wait) > 1:
                    waits = list(si.on_wait)
                    for j, w in enumerate(waits[:-1]):
                        out.append(mybir.InstEventSemaphore(
                            name=f"{i.name}_w{j}", engine=i.engine,
                            ins=[], outs=[],
                            sync_info=mybir.SyncInfo(on_wait=[w], on_update=[]),
                        ))
                    i.sync_info = mybir.SyncInfo(
                        on_wait=[waits[-1]], on_update=list(si.on_update))
                    n_split += 1
                out.append(i)
            b.instructions.clear()
            for x in out:
                b.instructions.append(x)
    return n_rc, n_split


def build_nc(Ms, Ns, Kd, BLK, ksplit):
    """One SPMD program; per-core data differs via in_maps."""
    MT = Ms // 128
    NB1 = ksplit // BLK                  # stage-1 blocks (BLK k's each)
    KG = Kd - ksplit                     # gathered k's (paired, 2 per gather)
    NPAIR = KG // 2
    PAIR_BLK = 4                         # pairs per gather-block
    NBG = NPAIR // PAIR_BLK
    IDXW = 2 * Ns // 16                  # idx columns per paired gather

    assert Ns * 2 <= MAX_GATHER_IDX or KG == 0
    assert MT % 4 == 0 or MT <= 4

    nc = bass.Bass()
    widx = nc.declare_dram_parameter("widx", [max(NBG, 1), 128, PAIR_BLK * IDXW], I16, False)
    abf = nc.declare_dram_parameter("abf", [Kd, Ms], BF16, False)
    wbf = nc.declare_dram_parameter("wbf", [max(ksplit, 1), Ns], BF16, False)
    lutT = nc.declare_dram_parameter("lutT", [256, 256], BF16, False)
    pcolb = nc.declare_dram_parameter("pcolb", [128, 2 * Ms], BF16, False)
    biasr = nc.declare_dram_parameter("biasr", [1, Ns], F32, False)
    ones = nc.declare_dram_parameter("ones", [1, 128], F32, False)
    out_d = nc.declare_dram_parameter("out", [Ms, Ns], F32, isOutput=True)

    with TileContext(nc) as tc:
        with (
            tc.tile_pool(name="const", bufs=1) as constp,
            tc.tile_pool(name="idx", bufs=3) as idxp,
            tc.tile_pool(name="arep", bufs=2) as arepp,
            tc.tile_pool(name="oneh", bufs=4) as onehp,
            tc.tile_pool(name="hbuf", bufs=6) as hbufp,
            tc.tile_pool(name="psacc", bufs=1, space="PSUM") as psaccp,
            tc.tile_pool(name="psh", bufs=2, space="PSUM") as pshp,
            tc.tile_pool(name="outs", bufs=2) as outp,
        ):
            nc.gpsimd.load_library(mlp)
            nidx_reg = nc.gpsimd.to_reg(2 * Ns)

            ones_t = constp.tile([1, 128], F32, tag="ones")
            nc.sync.dma_start(out=ones_t, in_=ones[:, :])
            bias_t = constp.tile([1, Ns], F32, tag="bias")
            nc.sync.dma_start(out=bias_t, in_=biasr[:, :])
            pcol_t = constp.tile([128, 2 * Ms], BF16, tag="pcolb")
            nc.sync.dma_start(out=pcol_t, in_=pcolb[:, :])
            # L^T rows q as partitions (2 chunks): lutsb[qc][q, p] = LUT[p, q]
            lutsb = []
            for qc in range(2):
                lt = constp.tile([128, 256], BF16, tag=f"lut{qc}")
                nc.sync.dma_start(out=lt, in_=lutT[qc * 128:(qc + 1) * 128, :])
                lutsb.append(lt)

            # 4 accumulator banks x 2 m-tile slots: m-tile mt -> bank mt%4, slot mt//4
            accs = [
                psaccp.tile([128, 2 * Ns], F32, tag=f"acc{b}", name=f"acc{b}")
                for b in range(4)
            ]

            def acc_slot(mt):
                return accs[mt % 4][:, (mt // 4) * Ns:(mt // 4 + 1) * Ns]

            def stage2(k, aoh, hslices):
                for half in range(2):
                    rhs = hslices[half]
                    for mt in range(MT):
                        lhsT = aoh[:, half, mt * 128:(mt + 1) * 128]
                        nc.tensor.matmul(
                            acc_slot(mt), lhsT, rhs,
                            start=(k == 0 and half == 0),
                            stop=False,
                        )

            def make_aoh(arep, kl):
                aoh = onehp.tile([128, 2, Ms], BF16, tag="aoh", name="aoh")
                for half in range(2):
                    nc.vector.tensor_tensor(
                        aoh[:, half, :], arep[:, kl, :],
                        pcol_t[:, half * Ms:(half + 1) * Ms],
                        mybir.AluOpType.is_equal,
                    )
                return aoh

            # ---- stage-1 region: H built on PE ----
            for b in range(NB1):
                arep = arepp.tile([128, BLK, Ms], BF16, tag="arep")
                nc.sync.dma_start(
                    out=arep,
                    in_=abf[b * BLK:(b + 1) * BLK, :].partition_broadcast(128),
                )
                wrep = arepp.tile([128, BLK, Ns], BF16, tag="wrep")
                nc.sync.dma_start(
                    out=wrep,
                    in_=wbf[b * BLK:(b + 1) * BLK, :].partition_broadcast(128),
                )
                for kl in range(BLK):
                    k = b * BLK + kl
                    aoh = make_aoh(arep, kl)
                    w1h = onehp.tile([128, 2, Ns], BF16, tag="w1h", name="w1h")
                    for half in range(2):
                        nc.vector.tensor_tensor(
                            w1h[:, half, :], wrep[:, kl, :],
                            pcol_t[:, half * Ms:half * Ms + Ns],
                            mybir.AluOpType.is_equal,
                        )
                    ht = hbufp.tile([128, 2, Ns], BF16, tag="ht", name="ht")
                    for pc in range(2):
                        hps = pshp.tile([128, Ns], F32, tag=f"hps{pc}", name=f"hps{pc}")
                        for qc in range(2):
                            nc.tensor.matmul(
                                hps, lutsb[qc][:, pc * 128:(pc + 1) * 128],
                                w1h[:, qc, :],
                                start=(qc == 0), stop=(qc == 1),
                            )
                        nc.scalar.copy(ht[:, pc, :], hps)
                    stage2(k, aoh, [ht[:, 0, :], ht[:, 1, :]])

            # ---- gather region: H gathered from lut.T, 2 k's per gather ----
            for b in range(NBG):
                wt = idxp.tile([128, PAIR_BLK * IDXW], I16, tag="widx")
                nc.sync.dma_start(out=wt, in_=widx[b])
                k0 = ksplit + b * PAIR_BLK * 2
                arep = arepp.tile([128, 2 * PAIR_BLK, Ms], BF16, tag="garep")
                nc.sync.dma_start(
                    out=arep,
                    in_=abf[k0:k0 + 2 * PAIR_BLK, :].partition_broadcast(128),
                )
                for pl in range(PAIR_BLK):
                    k = k0 + 2 * pl
                    hg = hbufp.tile([128, 2, 2 * Ns], BF16, tag="hg", name="hg")
                    nc.gpsimd.dma_gather(
                        hg[:], lutT[:, :], wt[:, pl * IDXW:(pl + 1) * IDXW],
                        2 * Ns, nidx_reg, 256, transpose=True, queue_num=0,
                    )
                    for j in range(2):
                        aoh = make_aoh(arep, 2 * pl + j)
                        stage2(k + j, aoh,
                               [hg[:, 0, j * Ns:(j + 1) * Ns],
                                hg[:, 1, j * Ns:(j + 1) * Ns]])

            for mt in range(MT):
                nc.tensor.matmul(acc_slot(mt), ones_t, bias_t, start=False, stop=True)

            for mt in range(MT):
                ot = outp.tile([128, Ns], F32, tag="ot")
                nc.vector.tensor_copy(ot, acc_slot(mt))
                nc.sync.dma_start(out=out_d[mt * 128:(mt + 1) * 128, :], in_=ot)
    return nc


def pack_gather_idx(codes_T, ksplit, pair_blk):
    """codes_T: [K, Ns]. For k >= ksplit, pack per-pair index streams
    [w^T[k], w^T[k+1]] (2*Ns indices), wrapped idx i -> [i%16, i//16] and
    replicated to the 8 Q7 groups; pairs grouped pair_blk per block.
    Returns [NBG, 128, pair_blk * 2*Ns//16] int16."""
    Kd, Ns = codes_T.shape
    g = codes_T[ksplit:]                         # [KG, Ns]
    pairs = g.reshape(-1, 2 * Ns)                # [NPAIR, 2*Ns]
    w = pairs.reshape(pairs.shape[0], -1, 16).transpose(0, 2, 1)  # [NP,16,2Ns/16]
    w = np.tile(w, (1, 8, 1))                    # [NP, 128, 2Ns/16]
    npair = w.shape[0]
    w = w.reshape(npair // pair_blk, pair_blk, 128, -1).transpose(0, 2, 1, 3)
    return np.ascontiguousarray(w).reshape(
        npair // pair_blk, 128, -1).astype(np.int16)


def make_in_maps(input, weight, bias, lut, Ms, Ns, Kd, BLK, ksplit, grid_n):
    a = np.clip(np.asarray(input).astype(np.int32), 0, 255)
    w = np.clip(np.asarray(weight).astype(np.int32), 0, 255)
    lutT = np.ascontiguousarray(np.asarray(lut).astype(np.float32).T).astype(
        ml_dtypes.bfloat16
    )
    pcolb = np.concatenate([
        np.repeat(np.arange(128)[:, None], Ms, axis=1),
        np.repeat(np.arange(128, 256)[:, None], Ms, axis=1),
    ], axis=1).astype(np.float32).astype(ml_dtypes.bfloat16)  # [128, 2*Ms]
    ones = np.ones((1, 128), np.float32)
    bias = np.asarray(bias).astype(np.float32)
    aT = np.ascontiguousarray(a.T).astype(np.float32).astype(ml_dtypes.bfloat16)

    in_maps = []
    for c in range(grid_n):
        wT = w[c * Ns:(c + 1) * Ns, :].T          # [K, Ns]
        in_maps.append({
            "widx": pack_gather_idx(wT, ksplit, 4),
            "abf": aT,
            "wbf": np.ascontiguousarray(wT[:max(ksplit, 1)]).astype(
                np.float32).astype(ml_dtypes.bfloat16),
            "lutT": lutT,
            "pcolb": pcolb,
            "biasr": bias[c * Ns:(c + 1) * Ns].reshape(1, Ns),
            "ones": ones,
        })
    return in_maps


_NC_CACHE = {}


def _get_nc(Ms, Ns, Kd, BLK, ksplit, compat=True):
    key = (Ms, Ns, Kd, BLK, ksplit)
    if key not in _NC_CACHE:
        nc = build_nc(Ms, Ns, Kd, BLK, ksplit)
        if compat:
            lower_extended_insts(nc)
            _walrus_compat(nc)
        _NC_CACHE[key] = nc
    return _NC_CACHE[key]


def kernel(input, weight, bias, lut, BLK=8, ksplit=KSPLIT, trace=False,
           trace_kwargs=None):
    Ms, Ns = M, N // GRID_N
    nc = _get_nc(Ms, Ns, K, BLK, ksplit)
    in_maps = make_in_maps(input, weight, bias, lut, Ms, Ns, K, BLK, ksplit,
                           GRID_N)
    res = run_bass_kernel_spmd(
        nc, in_maps, list(range(N_CORES)), trace=trace,
        trace_kwargs=trace_kwargs or {},
    )
    out = np.empty((M, N), np.float32)
    for c in range(GRID_N):
        out[:, c * Ns:(c + 1) * Ns] = res.results[c]["out"]
    if trace:
        kernel.last_exec_time_ns = res.exec_time_ns
    return out


kernel.last_exec_time_ns = None
